# revision 26
# baseline (speedup 1.0000x reference)
"""Multi-head attention (B=4, S=2048, D=1024, H=16) on 8 Trainium2 NeuronCores.

Sharding: core c -> batch c//2, head-group c%2 (8 heads = 512 dims each).
Each core computes qkv projection, softmax attention and its partial
out-projection (Megatron row-split of w_out); the host sums core pairs.

Design (per core):
- bf16 operands everywhere (fp32 PSUM accumulation); x resident in SBUF so
  all projections re-read it for free.
- One software-pipelined stream of 16 "windows" (head-pair x 512-query
  chunk): scores (two concurrent K=64 row-group matmuls) -> exp on the ACT
  engine -> PV matmuls of the PREVIOUS window ride along one window behind
  (lag-16 via 18 rotating pt buffers). The next pair's q/k projection and
  the out-projection run as PE filler inside the windows.
- Softmax denominators come free from an augmented ones-column in V; the
  reciprocal runs partition-parallel on a [128,4] reshape (DRAM-bounce) so
  the DVE never stalls the pipeline; normalization multiplies through a
  DMA-broadcast row.
- The exp stream on the ACT engine (256 x ~1.07us) is the pacing engine;
  PE work (~280us of matmuls) hides almost entirely underneath it.
"""

import numpy as np

B, S, D, H = 4, 2048, 1024, 16
HD = D // H          # 64
HG = H // 2          # 8 heads per core
DG = HG * HD         # 512 local head-cat dims
SCALE = HD ** -0.5   # folded into wq host-side
NCORES = 8

NSQT = S // 128      # 16 sk tiles
NDT = D // 128       # 8 contraction tiles
NPAIR = HG // 2      # 4 head pairs
VW = HD + 1          # 65: v columns + ones column per head
SQQ = 512            # sq chunk per window
NW = S // SQQ        # 4 windows (qu) per pair

_CACHE = {}


# --------------------------------------------------------------------------
# wait splitting: this toolchain's walrus rejects >1 sync wait per instruction
# on some paths; move excess semaphore waits onto same-engine NoOps.
# --------------------------------------------------------------------------
def _split_excess_waits(nc, max_waits=1):
    import bass_rust
    import concourse.mybir as mybir

    ctr = [0]
    for fn in nc.m.functions:
        for bb in fn.blocks:
            insts = list(bb.instructions)
            out = []
            changed = False
            for inst in insts:
                si = inst.sync_info
                waits = list(si.on_wait) if si is not None and si.on_wait else []
                sem_waits = [w for w in waits if w.sync_type == "semaphore"]
                other = [w for w in waits if w.sync_type != "semaphore"]
                budget = max_waits - len(other)
                if len(sem_waits) > budget and budget >= 1:
                    head, keep = sem_waits[:-budget], sem_waits[-budget:]
                    chunks = [
                        head[i : i + max_waits]
                        for i in range(0, len(head), max_waits)
                    ]
                    for ch in chunks:
                        nop = mybir.InstNoOp(
                            name=f"wsplit-{ctr[0]}",
                            opcode="NoOp",
                            engine=inst.engine,
                            ins=[],
                            outs=[],
                        )
                        nop.sync_info = bass_rust.SyncInfo(on_wait=ch, on_update=[])
                        ctr[0] += 1
                        out.append(nop)
                    inst.sync_info = bass_rust.SyncInfo(
                        on_wait=other + keep,
                        on_update=list(si.on_update) if si.on_update else [],
                    )
                    changed = True
                out.append(inst)
            if changed:
                bb.instructions = out


# --------------------------------------------------------------------------
# device program (identical on all 8 cores)
# --------------------------------------------------------------------------
def _build():
    import concourse.bass as bass
    import concourse.tile as tile
    import concourse.mybir as mybir

    F32 = mybir.dt.float32
    BF16 = mybir.dt.bfloat16
    EXP = mybir.ActivationFunctionType.Exp
    ts = bass.ts

    nc = bass.Bass()

    xT = nc.dram_tensor("xT", [D, S], BF16, kind="ExternalInput")
    wq = nc.dram_tensor("wq", [D, DG], BF16, kind="ExternalInput")
    wk = nc.dram_tensor("wk", [D, DG], BF16, kind="ExternalInput")
    wv = nc.dram_tensor("wv", [D, DG], BF16, kind="ExternalInput")
    bqk = nc.dram_tensor("bqk", [128, 8], F32, kind="ExternalInput")
    bv = nc.dram_tensor("bv", [128, DG], F32, kind="ExternalInput")
    wo = nc.dram_tensor("wo", [DG, D], BF16, kind="ExternalInput")
    bo = nc.dram_tensor("bo", [128, D], F32, kind="ExternalInput")
    outp = nc.dram_tensor("outp", [S, D], F32, kind="ExternalOutput")

    with tile.TileContext(nc) as tc:
        with (
            tc.tile_pool(name="wgt", bufs=1) as wgt,
            tc.tile_pool(name="xs", bufs=1) as xs,
            tc.tile_pool(name="bias", bufs=1) as bias_pool,
            tc.tile_pool(name="vt", bufs=1) as v_pool,
            tc.tile_pool(name="qk", bufs=1) as qk_pool,
            tc.tile_pool(name="oT", bufs=1) as oT_pool,
            tc.tile_pool(name="pt", bufs=1) as ptp,
            tc.tile_pool(name="pvs", bufs=2) as pvsp,
            tc.tile_pool(name="nrm", bufs=2) as nrm,
            tc.tile_pool(name="bcs", bufs=2) as bcsp,
            tc.tile_pool(name="ob", bufs=3) as obp,
            tc.tile_pool(name="rs", bufs=4, space="DRAM") as rsp,
            tc.tile_pool(name="sc", bufs=2, space="PSUM") as scp,
        ):
            # ---------------- static loads ----------------
            # DMA ordering puts the critical path to the first exp first:
            # biases, pair-0 slices of wq/wk, first x quarter, then wv (for
            # the v blocks), then the rest.
            bqk_t = bias_pool.tile([128, 8], F32)
            nc.sync.dma_start(bqk_t[:], bqk[:, :])
            bv_t = bias_pool.tile([128, DG], F32)
            nc.sync.dma_start(bv_t[:], bv[:, :])
            # d-major interleave: the d-th qk matmul of ci0 needs exactly
            # (wq[d] slice, wk[d] slice, x[d] quarter) — land them together so
            # the accumulation starts as soon as the first triple arrives.
            wq_t = [wgt.tile([128, DG], BF16, name=f"wq{d}", tag=f"wq{d}") for d in range(NDT)]
            wk_t = [wgt.tile([128, DG], BF16, name=f"wk{d}", tag=f"wk{d}") for d in range(NDT)]
            x_t = [xs.tile([128, S], BF16, name=f"x{d}", tag=f"x{d}") for d in range(NDT)]
            for d in range(NDT):
                nc.sync.dma_start(wq_t[d][:, 0:128], wq[ts(d, 128), 0:128])
                nc.sync.dma_start(wk_t[d][:, 0:128], wk[ts(d, 128), 0:128])
                nc.sync.dma_start(x_t[d][:, 0:512], xT[ts(d, 128), 0:512])
            wv_t = [wgt.tile([128, DG], BF16, name=f"wv{d}", tag=f"wv{d}") for d in range(NDT)]
            for d in range(NDT):
                nc.sync.dma_start(wv_t[d][:], wv[ts(d, 128), :])
            for q in range(1, 4):
                for d in range(NDT):
                    nc.sync.dma_start(
                        x_t[d][:, ts(q, 512)], xT[ts(d, 128), ts(q, 512)]
                    )
            for d in range(NDT):
                nc.sync.dma_start(wq_t[d][:, 128:DG], wq[ts(d, 128), 128:DG])
                nc.sync.dma_start(wk_t[d][:, 128:DG], wk[ts(d, 128), 128:DG])
            wo_t = [wgt.tile([128, D], BF16, name=f"wo{pp}", tag=f"wo{pp}") for pp in range(NPAIR)]
            for pp in range(NPAIR):
                nc.sync.dma_start(wo_t[pp][:], wo[ts(pp, 128), :])
            bo_t = bias_pool.tile([128, D], F32)
            nc.sync.dma_start(bo_t[:], bo[:, :])
            ones8_f = bias_pool.tile([128, 8], F32)
            nc.vector.memset(ones8_f[:], 1.0)
            ones8 = bias_pool.tile([128, 8], BF16)
            nc.vector.tensor_copy(ones8[:], ones8_f[:])

            v_t = [v_pool.tile([128, HG * VW], BF16, name=f"v{s}", tag=f"v{s}") for s in range(NSQT)]
            oT_t = [oT_pool.tile([128, S], BF16, name=f"oT{p}", tag=f"oT{p}") for p in range(NPAIR)]
            # 2-slot rotation: pair pr lives in slot pr%2
            qT_t = [qk_pool.tile([128, S], BF16, name=f"qT{i}", tag=f"qT{i}") for i in range(2)]
            kT_t = [qk_pool.tile([128, S], BF16, name=f"kT{i}", tag=f"kT{i}") for i in range(2)]

            # pt tiles: at most 17 are live at once (previous window's
            # unconsumed + current window's produced); 18 rotating tags give
            # a 2-slot reuse gap so the exp never waits on the deferred pv.
            pt_cur = [None] * NSQT   # written by current window's exps
            pt_prev = [None] * NSQT  # consumed by deferred pv matmuls
            pt_gidx = [0]

            # ---------------- emission helpers ----------------
            def emit_qk_ci(pr, ci):
                """project q,k for pair pr, column chunk ci, via one sc tile
                (gq in cols 0:512, gk in cols 512:1024)."""
                slot = pr % 2
                g = scp.tile([128, 2 * SQQ], F32, name="gqk", tag="sc")
                for d in range(NDT):
                    nc.tensor.matmul(
                        g[:, 0:SQQ], wq_t[d][:, ts(pr, 128)],
                        x_t[d][:, ts(ci, SQQ)],
                        start=(d == 0), stop=(d == NDT - 1),
                    )
                    nc.tensor.matmul(
                        g[:, SQQ : 2 * SQQ], wk_t[d][:, ts(pr, 128)],
                        x_t[d][:, ts(ci, SQQ)],
                        start=(d == 0), stop=(d == NDT - 1),
                    )
                nc.vector.tensor_scalar_add(
                    qT_t[slot][:, ts(ci, SQQ)], g[:, 0:SQQ],
                    bqk_t[:, pr : pr + 1],
                )
                nc.vector.tensor_scalar_add(
                    kT_t[slot][:, ts(ci, SQQ)], g[:, SQQ : 2 * SQQ],
                    bqk_t[:, 4 + pr : 5 + pr],
                )

            def emit_scores_exp(pr, qu, s):
                """scores for both heads of pair pr (concurrent row groups)
                + one exp into pt_cur[s]."""
                slot = pr % 2
                qs = slice(qu * SQQ, (qu + 1) * SQQ)
                sc = scp.tile([128, 2 * SQQ], F32, name="sc", tag="sc")
                nc.tensor.matmul(
                    sc[:, 0:SQQ], kT_t[slot][0:HD, ts(s, 128)],
                    qT_t[slot][0:HD, qs], start=True, stop=True,
                )
                nc.tensor.matmul(
                    sc[:, SQQ : 2 * SQQ], kT_t[slot][HD:128, ts(s, 128)],
                    qT_t[slot][HD:128, qs], start=True, stop=True,
                )
                tg = pt_gidx[0] % 18
                pt_gidx[0] += 1
                pt = ptp.tile([128, 2 * SQQ], BF16, name=f"pt{tg}", tag=f"pt{tg}")
                nc.scalar.activation(pt[:], sc[:], EXP)
                pt_cur[s] = pt

            def emit_pv(pr, s, pv0, pv1, cur=False):
                """pv accumulation, slot s. cur=True reads the current
                window's pt (inline lag-2 mode, used by pair 3)."""
                h0, h1 = 2 * pr, 2 * pr + 1
                pp_ = pt_cur[s] if cur else pt_prev[s]
                nc.tensor.matmul(
                    pv0[0:VW, :], v_t[s][:, h0 * VW : (h0 + 1) * VW],
                    pp_[:, 0:SQQ], start=(s == 0), stop=(s == NSQT - 1),
                )
                nc.tensor.matmul(
                    pv1[0:VW, :], v_t[s][:, h1 * VW : (h1 + 1) * VW],
                    pp_[:, SQQ : 2 * SQQ], start=(s == 0), stop=(s == NSQT - 1),
                )

            def emit_norm(pr, qu, pv0, pv1):
                """Normalize pv by the denominator row into oT.

                The denominator row goes PSUM->DRAM immediately (parallel to
                the PSUM->SBUF copies), is reshaped to [128,4] so the DVE
                reciprocal runs partition-parallel (~0.15us instead of 3.3us
                on a single partition), then is scattered back and broadcast.
                Keeping the DVE chain short matters: the next window's pv
                matmuls WAR-wait on the copies, so anything queued before
                them on the DVE FIFO stalls the PE."""
                emit_norm_p2(pr, qu, emit_norm_p1(pr, qu, pv0, pv1))

            _np1 = [0]

            def emit_norm_p1(pr, qu, pv0, pv1):
                """PSUM->SBUF copies + denominator-row DMA (frees the pv
                banks fast; nothing slow ahead of them on the DVE FIFO)."""
                k = _np1[0] % 2
                _np1[0] += 1
                rs_t, pvs_t = [], []
                for hh, pvx in ((0, pv0), (1, pv1)):
                    pvs = pvsp.tile([VW, SQQ], F32, name=f"pvs{hh}", tag=f"pvs{k}{hh}")
                    nc.vector.tensor_copy(pvs[:], pvx[0:VW, :])
                    pvs_t.append(pvs)
                for hh in (0, 1):
                    rs = rsp.tile([1, SQQ], F32, name=f"rs{hh}", tag=f"rs{k}{hh}")
                    nc.sync.dma_start(rs[:], pvs_t[hh][HD : HD + 1, :])
                    rs_t.append(rs)
                return k, rs_t, pvs_t

            def emit_norm_p2(pr, qu, state):
                """Partition-parallel reciprocal via DRAM reshape, broadcast,
                and the final normalize into oT."""
                k, rs_t, pvs_t = state
                qs = slice(qu * SQQ, (qu + 1) * SQQ)
                JW = SQQ // 128  # 4
                bcs_t = []
                for hh in (0, 1):
                    dn = nrm.tile([128, JW], F32, name=f"dn{hh}", tag=f"dn{k}{hh}")
                    nc.sync.dma_start(
                        dn[:], rs_t[hh][:].rearrange("o (p j) -> (o p) j", j=JW)
                    )
                    rq = nrm.tile([128, JW], F32, name=f"rq{hh}", tag=f"rq{k}{hh}")
                    nc.vector.reciprocal(rq[:], dn[:])
                    rt = rsp.tile([1, SQQ], F32, name=f"rt{hh}", tag=f"rt{k}{hh}")
                    nc.sync.dma_start(
                        rt[:].rearrange("o (p j) -> (o p) j", j=JW), rq[:]
                    )
                    bcs = bcsp.tile([HD, SQQ], F32, name=f"bcs{hh}", tag=f"bcs{k}{hh}")
                    nc.sync.dma_start(bcs[:], rt[:].broadcast_to([HD, SQQ]))
                    bcs_t.append(bcs)
                for hh, row in ((0, 0), (1, HD)):
                    nc.vector.tensor_mul(
                        oT_t[pr][row : row + HD, qs], pvs_t[hh][0:HD, :], bcs_t[hh][:]
                    )

            def emit_v_block(vblk, vpool):
                """one v sub-block: sg = vblk//2, si pair = vblk%2."""
                sg, half = divmod(vblk, 2)
                psv = [
                    vpool.tile([128, DG], F32, name="psv", tag=t)
                    for t in ("vA", "vB")
                ]
                for d in range(NDT):
                    for j, si in enumerate((2 * half, 2 * half + 1)):
                        nc.tensor.matmul(
                            psv[j][:],
                            x_t[d][:, sg * SQQ + si * 128 : sg * SQQ + (si + 1) * 128],
                            wv_t[d][:],
                            start=(d == 0), stop=(d == NDT - 1),
                        )
                for j, si in enumerate((2 * half, 2 * half + 1)):
                    s = 4 * sg + si
                    vap = v_t[s][:].rearrange("p (h e) -> p h e", e=VW)
                    nc.vector.tensor_add(
                        vap[:, :, 0:HD],
                        psv[j][:].rearrange("p (h e) -> p h e", e=HD),
                        bv_t[:].rearrange("p (h e) -> p h e", e=HD),
                    )
                    nc.vector.tensor_copy(vap[:, :, HD : HD + 1], ones8[:, :, None])

            def emit_outproj(t, j2, po_tag):
                po = gpool.tile([128, SQQ], F32, name="po", tag=po_tag)
                for pp in range(NPAIR):
                    nc.tensor.matmul(
                        po[:], oT_t[pp][:, ts(t, 128)], wo_t[pp][:, ts(j2, SQQ)],
                        start=(pp == 0), stop=(pp == NPAIR - 1),
                    )
                ob = obp.tile([128, SQQ], F32, tag="ob")
                nc.vector.tensor_add(ob[:], po[:], bo_t[:, ts(j2, SQQ)])
                nc.sync.dma_start(outp[ts(t, 128), ts(j2, SQQ)], ob[:])

            # ---------------- prephase + pair 0 window 0 ----------------
            # qk0 ci-blocks and v sub-blocks interleaved with the first
            # window's scores/exp so the ACT engine starts early.
            with tc.tile_pool(name="vps", bufs=2, space="PSUM") as vpool:
                emit_qk_ci(0, 0)
                for blk in range(4):  # blk = ci index 0..3
                    if blk > 0:
                        emit_qk_ci(0, blk)
                    lo, hi = 4 * blk, 4 * blk + 4
                    mid = lo + 2
                    for s in range(lo, mid):
                        emit_scores_exp(0, 0, s)
                    emit_v_block(2 * blk, vpool)
                    for s in range(mid, hi):
                        emit_scores_exp(0, 0, s)
                    emit_v_block(2 * blk + 1, vpool)

            # ---------------- main pipelined windows ----------------
            with (
                tc.tile_pool(name="gp", bufs=1, space="PSUM") as gpool,
                tc.tile_pool(name="pvp", bufs=1, space="PSUM") as pvp,
            ):
                # filler schedule: pair pr -> pr+1 q/k projection, emitted as
                # per-slot matmuls. pair 1: 64 MMs over pr=0 windows 1..3
                # (48 slots); pairs 2,3: 64 MMs over 64 slots.
                def filler_plan(pr, qu):
                    """list of (ci, d, is_k, is_last) per slot index 0..15"""
                    if pr == 3:
                        return {}
                    if pr == 0:
                        if qu == 0:
                            return {}
                        base = (qu - 1) * 16
                        nper, tot = 2, 48
                    else:
                        base = qu * 16
                        nper, tot = 1, 64
                    plan = {}
                    for i in range(16):
                        items = []
                        for j in range(nper):
                            k = (base + i) * nper + j if nper > 1 else base + i
                            if k < 64:
                                ci, r = divmod(k, 16)
                                d, is_k = divmod(r, 2)
                                items.append((ci, d, is_k, r == 15))
                        plan[i] = items
                    return plan

                g_live = {}  # tag -> (tile, pr, ci) accumulation in flight

                def emit_filler(pr_next, ci, d, is_k, is_last):
                    slot = pr_next % 2
                    tag = "g1" if is_k else "g0"
                    if d == 0 and not is_k:
                        g_live["g0"] = gpool.tile([128, SQQ], F32, name="gq", tag="g0")
                        g_live["g1"] = gpool.tile([128, SQQ], F32, name="gk", tag="g1")
                    g = g_live[tag]
                    src = wk_t[d] if is_k else wq_t[d]
                    nc.tensor.matmul(
                        g[:], src[:, ts(pr_next, 128)], x_t[d][:, ts(ci, SQQ)],
                        start=(d == 0), stop=(d == NDT - 1),
                    )
                    if is_last:
                        nc.vector.tensor_scalar_add(
                            qT_t[slot][:, ts(ci, SQQ)], g_live["g0"][:],
                            bqk_t[:, pr_next : pr_next + 1],
                        )
                        nc.vector.tensor_scalar_add(
                            kT_t[slot][:, ts(ci, SQQ)], g_live["g1"][:],
                            bqk_t[:, 4 + pr_next : 5 + pr_next],
                        )

                # deferred-window bookkeeping
                def roll_windows():
                    for s in range(NSQT):
                        pt_prev[s] = pt_cur[s]
                        pt_cur[s] = None

                prev = (0, 0)  # window whose pv/norm runs now
                roll_windows()

                # All windows run pv at lag-16 (previous window's pts).
                # Out-projection groups ride pair-3 windows from slot 6 on
                # (the norm chain feeding oT has ~10us of DVE+DMA latency).
                windows = [(pr, qu) for pr in range(NPAIR) for qu in range(NW)]
                for pr, qu in windows[1:]:
                    fplan = filler_plan(pr, qu)
                    pv0 = pvp.tile([128, SQQ], F32, name="pv0", tag="pv0")
                    pv1 = pvp.tile([128, SQQ], F32, name="pv1", tag="pv1")
                    op_list = []
                    if pr == 3 and qu >= 2:
                        gsel = qu - 2
                        op_list = [
                            (4 * gsel + t, j2) for t in range(4) for j2 in range(2)
                        ]
                    for s in range(NSQT):
                        # ready work (pv/fillers/out-proj) first: the scores
                        # matmul WAR-waits on exp(s-2), and putting it at the
                        # slot head would block the whole PE FIFO behind that
                        # wait, micro-idling the PE and re-throttling HAM.
                        emit_pv(prev[0], s, pv0, pv1)
                        for (ci, d, is_k, is_last) in fplan.get(s, []):
                            emit_filler(pr + 1, ci, d, is_k, is_last)
                        if op_list and s >= 6 and s % 2 == 0:
                            t, j2 = op_list.pop(0)
                            emit_outproj(t, j2, "g0" if (s // 2) % 2 == 0 else "g1")
                            if s >= 10 and op_list:
                                t, j2 = op_list.pop(0)
                                emit_outproj(t, j2, "g1" if (s // 2) % 2 == 0 else "g0")
                        emit_scores_exp(pr, qu, s)
                    emit_norm(prev[0], prev[1], pv0, pv1)
                    roll_windows()
                    prev = (pr, qu)

                # ---------------- tail ----------------
                pv0 = pvp.tile([128, SQQ], F32, name="pv0", tag="pv0")
                pv1 = pvp.tile([128, SQQ], F32, name="pv1", tag="pv1")
                for s in range(NSQT):
                    emit_pv(3, s, pv0, pv1)
                # out-proj group 2 only needs norm(3,2) (already done): emit it
                # before norm(3,3) so the PE chews on it while the DVE+DMA runs
                # the last normalization chain.
                for t in range(4):
                    for j2 in range(2):
                        emit_outproj(8 + t, j2, "g0" if j2 == 0 else "g1")
                emit_norm(3, 3, pv0, pv1)
                for t in range(4):
                    for j2 in range(2):
                        emit_outproj(12 + t, j2, "g0" if j2 == 0 else "g1")

    _split_excess_waits(nc, max_waits=1)
    return nc


def _get_nc():
    if "nc" not in _CACHE:
        _CACHE["nc"] = _build()
    return _CACHE["nc"]


# --------------------------------------------------------------------------
# host entry point
# --------------------------------------------------------------------------
def _shard_inputs(x, w_qkv, b_qkv, w_out, b_out):
    from ml_dtypes import bfloat16

    f = np.float32
    x = np.asarray(x, f)
    w_qkv = np.asarray(w_qkv, f)
    b_qkv = np.asarray(b_qkv, f)
    w_out = np.asarray(w_out, f)
    b_out = np.asarray(b_out, f)
    in_maps = []
    for c in range(NCORES):
        b, g = divmod(c, 2)
        cols = slice(DG * g, DG * (g + 1))
        wq_c = np.ascontiguousarray(w_qkv[:, 0 * D :][:, cols][:, :DG]) * np.float32(SCALE)
        wk_c = np.ascontiguousarray(w_qkv[:, D : 2 * D][:, cols])
        wv_c = np.ascontiguousarray(w_qkv[:, 2 * D :][:, cols])
        bq_c = (b_qkv[0 * D : 1 * D][cols] * np.float32(SCALE)).reshape(4, 128).T
        bk_c = b_qkv[D : 2 * D][cols].reshape(4, 128).T
        bqk_c = np.ascontiguousarray(np.concatenate([bq_c, bk_c], axis=1), f)
        bv_c = np.ascontiguousarray(np.tile(b_qkv[2 * D :][cols], (128, 1)), f)
        wo_c = np.ascontiguousarray(w_out[DG * g : DG * (g + 1), :])
        bo_c = (
            np.ascontiguousarray(np.tile(b_out, (128, 1)), f)
            if g == 0
            else np.zeros((128, D), f)
        )
        in_maps.append(
            {
                "xT": np.ascontiguousarray(x[b].T).astype(bfloat16),
                "wq": wq_c.astype(bfloat16),
                "wk": wk_c.astype(bfloat16),
                "wv": wv_c.astype(bfloat16),
                "bqk": bqk_c,
                "bv": bv_c,
                "wo": wo_c.astype(bfloat16),
                "bo": bo_c,
            }
        )
    return in_maps


def _patch_ldw_opt():
    """Flip walrus --enable-ldw-opt to true (dedupe repeated LDWEIGHTS for
    consecutive same-stationary matmuls). Off by default: bf16 LDWEIGHTS
    (fast-weight-load path) are rejected by walrus when the opt is on."""
    import os
    if os.environ.get("KERNEL_LDW_OPT", "0") != "1":
        return
    if _CACHE.get("ldw_patched"):
        return
    import concourse.bass_utils as bu

    orig = bu.run_command

    def run_command_ldw(argv, **kwargs):
        argv = [a.replace("--enable-ldw-opt=false", "--enable-ldw-opt=true")
                if isinstance(a, str) else a for a in argv]
        return orig(argv, **kwargs)

    bu.run_command = run_command_ldw
    _CACHE["ldw_patched"] = True


def kernel(x, w_qkv, b_qkv, w_out, b_out, _trace=False, _trace_kwargs=None):
    from concourse.bass_utils import run_bass_kernel_spmd

    _patch_ldw_opt()
    nc = _get_nc()
    in_maps = _shard_inputs(x, w_qkv, b_qkv, w_out, b_out)
    kw = {}
    if _trace:
        kw["trace"] = True
        kw.update(_trace_kwargs or {})
    res = run_bass_kernel_spmd(nc, in_maps, core_ids=list(range(NCORES)), **kw)
    _CACHE["last_result"] = res
    parts = [r["outp"] for r in res.results]
    out = np.stack([parts[2 * b] + parts[2 * b + 1] for b in range(B)])
    return np.ascontiguousarray(out, np.float32)
